# revision 5
# baseline (speedup 1.0000x reference)
"""Trainium2 Bass kernel for nn_Decoder_44882408243654.

LSTMCell(H=512) recurrence, B=256, 326 steps where from step 1 on the cell
input IS the hidden state, plus a deferred MLP head (512->512 relu ->9).

Strategy:
- Data-parallel over batch: 8 cores x 32 rows, weights replicated, no
  collectives (the recurrence is strictly sequential in time).
- All weights + the full hidden-state history stay SBUF-resident; only 4 DMAs
  per core (three const blobs in, y out).
- Steps >=1 fold W_ih+W_hh into one matrix. Batch=32 gates matmul is packed
  4-wide into the PE array via tile_position col-tiling: PSUM gates tile is
  [128, 512] with partition groups = [o, i, f, g] gates.
- sigmoid(x) == (tanh(x/2)+1)/2: one tanh table set serves all activations;
  the +1/x0.5 algebra is folded into scalar_tensor_tensor ops and the
  doubled representations C2=2c, H2=2h (weight scaling folded on host).
- h is produced directly in transposed (lhsT) layout via PE transposes of o
  and c2, so the next step's matmul needs no extra data movement.
- The MLP head is batched over all 326 steps after the recurrence (full
  128-partition matmuls against the stored H2 history).
"""
import sys
sys.path.insert(0, "/opt/trn_rl_repo")

import numpy as np
import ml_dtypes
from contextlib import ExitStack

import concourse.bass as bass
import concourse.bacc as bacc_mod
import concourse.tile as tile
from concourse import mybir
from concourse.bass_utils import run_bass_kernel_spmd

F32 = mybir.dt.float32
BF16 = mybir.dt.bfloat16
AF = mybir.ActivationFunctionType
ALU = mybir.AluOpType

B, H, OUT = 256, 512, 9
STEPS = 326            # total outputs (1 initial + 325 recurrent)
NCORES = 8
BL = B // NCORES       # 32 rows per core
G4 = 4 * H             # 2048

# main bf16 blob column offsets
O_WCT = 0                      # 4 k-chunks x [128, 2048] (recurrent, 0.5*Wc)
O_BROW = O_WCT + 4 * G4        # row 0: bias row [2048]
O_ONES = O_BROW + G4           # row 0: ones [32]
O_W1T = O_ONES + 32            # W1 lhsT: 4 k-chunks x [128, 512]
O_W2T = O_W1T + 4 * 512        # W2 lhsT: 4 k-chunks x [128, 9] (16-col pad)
NB = O_W2T + 4 * 16

# step-0 bf16 blob (released after the recurrence)
S_WIH = 0                      # step-0 x weights, 4 x [128, 2048]
S_WHH = S_WIH + 4 * G4         # step-0 h weights
S_X0T = S_WHH + 4 * G4         # [128, 128] x transposed (lhsT layout)
S_H0T = S_X0T + 128            # [128, 128] hx0 transposed
NS = S_H0T + 128

# f32 blob column offsets
F_ID = 0                       # [128, 32] tiled identity(32)
F_C2 = 32                      # rows 64:96 = 2*cx0  [32, 512]
F_B1 = F_C2 + 512              # [128, 4] b1 per-partition cols per m-chunk
F_B2 = F_B1 + 4                # rows 0:9 col = b2
NF = F_B2 + 1

_BUILT = {}


def _build(steps):
    """Build + finalize the SPMD bass program for `steps` outputs."""
    nt = steps * BL
    nc = bacc_mod.Bacc()
    d_bb = nc.declare_dram_parameter("blob_b", [128, NB], BF16, isOutput=False)
    d_s0 = nc.declare_dram_parameter("blob_s0", [128, NS], BF16, isOutput=False)
    d_bf = nc.declare_dram_parameter("blob_f", [128, NF], F32, isOutput=False)
    d_yt = nc.declare_dram_parameter("yt", [OUT, nt], F32, isOutput=True)

    with tile.TileContext(nc) as tc, ExitStack() as ctx:
        const = ctx.enter_context(tc.tile_pool(name="const", bufs=1))
        hist = ctx.enter_context(tc.tile_pool(name="hist", bufs=1))

        bb = const.tile([128, NB], BF16)
        bf = const.tile([128, NF], F32)
        nc.sync.dma_start(bb[:], d_bb[:])
        nc.sync.dma_start(bf[:], d_bf[:])

        wct = [bb[:, O_WCT + G4 * k:O_WCT + G4 * (k + 1)] for k in range(4)]
        brow = bb[0:1, O_BROW:O_BROW + G4]
        ones = bb[0:1, O_ONES:O_ONES + 32]
        w1t = [bb[:, O_W1T + 512 * k:O_W1T + 512 * (k + 1)] for k in range(4)]
        w2t = [bb[:, O_W2T + 16 * k:O_W2T + 16 * k + OUT] for k in range(4)]
        id_t = bf[:, F_ID:F_ID + 32]
        c2f = bf[:, F_C2:F_C2 + 512]     # rows 64:96 = C2 state (in place)
        b1c = bf[:, F_B1:F_B1 + 4]
        b2c = bf[0:OUT, F_B2:F_B2 + 1]

        HT = hist.tile([128, nt * 4], BF16)   # H2 history, lhsT layout

        # ---------------- recurrence ----------------
        with (
            tc.tile_pool(name="s0pool", bufs=1) as s0pool,
            tc.tile_pool(name="work", bufs=2) as work,
            tc.tile_pool(name="gps", bufs=2, space="PSUM") as gps,
            tc.tile_pool(name="tps", bufs=2, space="PSUM") as tps,
        ):
            s0 = s0pool.tile([128, NS], BF16)
            nc.sync.dma_start(s0[:], d_s0[:])
            wih = [s0[:, S_WIH + G4 * k:S_WIH + G4 * (k + 1)] for k in range(4)]
            whh = [s0[:, S_WHH + G4 * k:S_WHH + G4 * (k + 1)] for k in range(4)]
            x0t = s0[:, S_X0T:S_X0T + 128]
            h0t = s0[:, S_H0T:S_H0T + 128]

            tc.strict_bb_all_engine_barrier()

            for t in range(steps):
                gates = gps.tile([128, 512], F32, name="gates")
                # accumulation per col-group jg (gate order o,i,f,g):
                # bias row (K=1) then K-chunk matmuls
                if t == 0:
                    terms = [(x0t, wih), (h0t, whh)]
                else:
                    base = 128 * (t - 1)
                    hprev = HT[:, base:base + 128]
                    terms = [(hprev, wct)]
                for jg in range(4):
                    oap = gates[32 * jg:32 * jg + 32, :]
                    nc.tensor.matmul(oap, ones, brow[:, 512 * jg:512 * (jg + 1)],
                                     start=True, stop=False,
                                     tile_position=(0, 32 * jg))
                n_terms = len(terms)
                for ti, (lhs, rhs) in enumerate(terms):
                    for k in range(4):
                        last = (ti == n_terms - 1) and (k == 3)
                        for jg in range(4):
                            oap = gates[32 * jg:32 * jg + 32, :]
                            nc.tensor.matmul(
                                oap, lhs[:, 32 * k:32 * k + 32],
                                rhs[k][:, 512 * jg:512 * (jg + 1)],
                                start=False, stop=last,
                                tile_position=(0, 32 * jg))

                # activations: tanh(0.5 x) on o,i,f; tanh(x) on g (in place)
                t_sb = work.tile([96, 512], F32, name="t_sb")
                nc.scalar.activation(t_sb[:], gates[0:96, :], AF.Tanh,
                                     bias=0.0, scale=0.5)
                nc.scalar.activation(gates[96:128, :], gates[96:128, :], AF.Tanh)

                # u = (ti+1)*g  -> psum gates[32:64] (i rows dead)
                nc.vector.scalar_tensor_tensor(
                    gates[32:64, :], t_sb[32:64, :], 1.0, gates[96:128, :],
                    op0=ALU.add, op1=ALU.mult)
                # w = (tf+1)*C2 (both SBUF @base64)
                w_sb = work.tile([96, 512], F32, name="w_sb")
                nc.vector.scalar_tensor_tensor(
                    w_sb[64:96, :], t_sb[64:96, :], 1.0, c2f[64:96, :],
                    op0=ALU.add, op1=ALU.mult)
                # C2' = 0.5*w + u
                nc.vector.scalar_tensor_tensor(
                    c2f[64:96, :], w_sb[64:96, :], 0.5, gates[32:64, :],
                    op0=ALU.mult, op1=ALU.add)

                # transposes into PSUM: o rows (base 0), C2' (base 64)
                toT = tps.tile([128, 128], F32, name="toT")
                for j in range(4):
                    nc.tensor.transpose(toT[:, 32 * j:32 * j + 32],
                                        t_sb[0:32, 128 * j:128 * (j + 1)],
                                        id_t[0:32, :])
                c2T = tps.tile([128, 128], F32, name="c2T")
                for j in range(4):
                    nc.tensor.transpose(c2T[:, 32 * j:32 * j + 32],
                                        c2f[64:96, 128 * j:128 * (j + 1)],
                                        id_t[64:96, :])

                tcT = work.tile([128, 128], F32, name="tcT")
                nc.scalar.activation(tcT[:], c2T[:], AF.Tanh, bias=0.0, scale=0.5)
                # H2_t = (toT+1)*tcT  -> bf16 history slice (lhsT layout)
                nc.vector.scalar_tensor_tensor(
                    HT[:, 128 * t:128 * (t + 1)], toT[:], 1.0, tcT[:],
                    op0=ALU.add, op1=ALU.mult)

        # ---------------- batched MLP head ----------------
        # z.T = relu(0.5*W1 @ H2.T + b1)  [512, nt] ; y.T = W2 @ z.T + b2
        with (
            tc.tile_pool(name="ypool", bufs=1) as ypool,
            tc.tile_pool(name="zwork", bufs=2) as zwork,
            tc.tile_pool(name="zps", bufs=2, space="PSUM") as zpsp,
            tc.tile_pool(name="yps", bufs=2, space="PSUM") as ypsp,
        ):
            yT = ypool.tile([OUT, nt], F32)
            CT = 512           # columns (t,b) per tile = 16 time steps
            n_ct = (nt + CT - 1) // CT
            for ct in range(n_ct):
                c0 = ct * CT
                w = min(CT, nt - c0)
                tb0 = c0 // BL          # first t index in this tile
                ntb = w // BL           # t steps in this tile
                hblk = HT[:, 128 * tb0:128 * (tb0 + ntb)]
                hblk = hblk.rearrange("p (t x) -> p t x", x=128)
                z_sb = []
                for m in range(4):
                    zps = zpsp.tile([128, CT], F32, name="zps")
                    for k in range(4):
                        rhs = hblk[:, :, 32 * k:32 * k + 32]
                        nc.tensor.matmul(zps[:, 0:w],
                                         w1t[k][:, 128 * m:128 * (m + 1)],
                                         rhs, start=(k == 0), stop=(k == 3))
                    zt = zwork.tile([128, CT], BF16, name="z_sb", tag=f"z{m}")
                    # relu(x + b1), split between DVE and ACT to balance load
                    if m % 2 == 0:
                        nc.vector.tensor_scalar(
                            zt[:, 0:w], zps[:, 0:w], b1c[:, m:m + 1], 0.0,
                            ALU.add, ALU.max)
                    else:
                        nc.scalar.activation(zt[:, 0:w], zps[:, 0:w], AF.Relu,
                                             bias=b1c[:, m:m + 1], scale=1.0)
                    z_sb.append(zt)
                yps = ypsp.tile([OUT, CT], F32, name="yps")
                for k in range(4):
                    nc.tensor.matmul(yps[:, 0:w], w2t[k], z_sb[k][:, 0:w],
                                     start=(k == 0), stop=(k == 3))
                nc.scalar.activation(yT[0:OUT, c0:c0 + w], yps[:, 0:w],
                                     AF.Identity, bias=b2c, scale=1.0)

            nc.sync.dma_start(d_yt[:], yT[:])

    nc.finalize()
    return nc


def _prep_host(x, hx0, cx0, W_ih, W_hh, b_ih, b_hh, W1, b1, W2, b2):
    """Build the per-core input blobs (all weight algebra folded here)."""
    perm = [3, 0, 1, 2]  # pytorch gate blocks i,f,g,o -> device order o,i,f,g

    def reorder(wm):
        blocks = wm.reshape(4, H, -1) if wm.ndim == 2 else wm.reshape(4, H)
        return np.concatenate([blocks[p] for p in perm], axis=0)

    Wih_r = reorder(W_ih)            # [2048, 512]
    Whh_r = reorder(W_hh)
    bc_r = reorder(b_ih + b_hh)      # [2048]
    Wc_r = 0.5 * (Wih_r + Whh_r)     # input is H2=2h

    def kchunksT(Wm):  # -> [4, 128, 2048], rhs layout per k-chunk
        return np.stack([Wm[:, 128 * j:128 * (j + 1)].T for j in range(4)])

    blob_b = np.zeros((128, NB), np.float32)
    wctT = kchunksT(Wc_r)
    for k in range(4):
        blob_b[:, O_WCT + G4 * k:O_WCT + G4 * (k + 1)] = wctT[k]
    blob_b[0, O_BROW:O_BROW + G4] = bc_r
    blob_b[0, O_ONES:O_ONES + 32] = 1.0
    W1T = (0.5 * W1).T               # [512, 512]; input is H2=2h
    for k in range(4):
        blob_b[:, O_W1T + 512 * k:O_W1T + 512 * (k + 1)] = \
            W1T[128 * k:128 * (k + 1), :]
        blob_b[:, O_W2T + 16 * k:O_W2T + 16 * k + OUT] = \
            W2.T[128 * k:128 * (k + 1), :]
    blob_b = blob_b.astype(ml_dtypes.bfloat16)

    blob_s0 = np.zeros((128, NS), np.float32)
    wihT = kchunksT(Wih_r)
    whhT = kchunksT(Whh_r)
    for k in range(4):
        blob_s0[:, S_WIH + G4 * k:S_WIH + G4 * (k + 1)] = wihT[k]
        blob_s0[:, S_WHH + G4 * k:S_WHH + G4 * (k + 1)] = whhT[k]

    def lhsT128(mat):  # [32, 512] -> [128, 128] lhsT tile layout
        o = np.zeros((128, 128), np.float32)
        for j in range(4):
            o[:, 32 * j:32 * j + 32] = mat[:, 128 * j:128 * (j + 1)].T
        return o

    blob_f0 = np.zeros((128, NF), np.float32)
    blob_f0[:, F_ID:F_ID + 32] = np.tile(np.eye(32, dtype=np.float32), (4, 1))
    blob_f0[:, F_B1:F_B1 + 4] = b1.reshape(4, 128).T
    blob_f0[0:OUT, F_B2] = b2

    in_maps = []
    for ci in range(NCORES):
        sl = slice(BL * ci, BL * (ci + 1))
        s0c = blob_s0.copy()
        s0c[:, S_X0T:S_X0T + 128] = lhsT128(x[sl])
        s0c[:, S_H0T:S_H0T + 128] = lhsT128(hx0[sl])
        bfc = blob_f0.copy()
        bfc[64:96, F_C2:F_C2 + 512] = 2.0 * cx0[sl]
        in_maps.append({
            "blob_b": blob_b,
            "blob_s0": s0c.astype(ml_dtypes.bfloat16),
            "blob_f": bfc,
        })
    return in_maps


class _Runner:
    """Cached jit(shard_map(bass_exec)) runner, mirrors run_bass_via_pjrt
    but reusable across calls and benchable with device-resident inputs."""

    def __init__(self, nc):
        import jax
        from jax.sharding import Mesh, PartitionSpec, NamedSharding
        from jax.experimental.shard_map import shard_map
        from concourse import bass2jax, mybir as _mb
        bass2jax.install_neuronx_cc_hook()
        self.jax = jax
        self.nc = nc
        part_name = (nc.partition_id_tensor.name
                     if nc.partition_id_tensor else None)
        in_names, out_names, out_avals, zero_outs = [], [], [], []
        for alloc in nc.m.functions[0].allocations:
            if not isinstance(alloc, _mb.MemoryLocationSet):
                continue
            name = alloc.memorylocations[0].name
            if alloc.kind == "ExternalInput":
                if name != part_name:
                    in_names.append(name)
            elif alloc.kind == "ExternalOutput":
                out_names.append(name)
                shape = tuple(alloc.tensor_shape)
                dtype = _mb.dt.np(alloc.dtype)
                out_avals.append(jax.core.ShapedArray(shape, dtype))
                zero_outs.append(np.zeros(shape, dtype))
        self.in_names, self.out_names = in_names, out_names
        self.zero_outs = zero_outs
        all_names = in_names + out_names
        if part_name is not None:
            all_names = all_names + [part_name]
        all_names = tuple(all_names)

        def _body(*args):
            operands = list(args)
            if part_name is not None:
                operands.append(bass2jax.partition_id_tensor())
            outs = bass2jax._bass_exec_p.bind(
                *operands, out_avals=tuple(out_avals), in_names=all_names,
                out_names=tuple(out_names),
                lowering_input_output_aliases=(),
                sim_require_finite=True, sim_require_nnan=True, nc=nc)
            return tuple(outs)

        devices = jax.devices()[:NCORES]
        self.mesh = Mesh(np.asarray(devices), ("core",))
        spec = PartitionSpec("core")
        self.sharding = NamedSharding(self.mesh, spec)
        n_args = len(in_names) + len(out_names)
        self.fn = jax.jit(
            shard_map(_body, mesh=self.mesh, in_specs=(spec,) * n_args,
                      out_specs=(spec,) * len(out_names), check_rep=False),
            keep_unused=True)

    def _concat(self, in_maps):
        return [np.concatenate([np.asarray(m[n]) for m in in_maps], axis=0)
                for n in self.in_names] + \
               [np.zeros((NCORES * z.shape[0], *z.shape[1:]), z.dtype)
                for z in self.zero_outs]

    def run(self, in_maps):
        outs = self.fn(*self._concat(in_maps))
        res = []
        for c in range(NCORES):
            d = {}
            for i, n in enumerate(self.out_names):
                a = np.asarray(outs[i])
                d[n] = a.reshape(NCORES, a.shape[0] // NCORES, *a.shape[1:])[c]
            res.append(d)
        return res

    def bench(self, in_maps, iters=5):
        jax = self.jax
        dev_args = [jax.device_put(a, self.sharding)
                    for a in self._concat(in_maps)]
        for a in dev_args:
            a.block_until_ready()
        outs = self.fn(*dev_args)          # warm
        jax.block_until_ready(outs)
        import time
        times = []
        for _ in range(iters):
            t0 = time.perf_counter()
            outs = self.fn(*dev_args)
            jax.block_until_ready(outs)
            times.append(time.perf_counter() - t0)
        return min(times), times


_RUNNERS = {}


def _get_runner(steps):
    if steps not in _RUNNERS:
        if steps not in _BUILT:
            _BUILT[steps] = _build(steps)
        _RUNNERS[steps] = _Runner(_BUILT[steps])
    return _RUNNERS[steps]


def kernel(x, hx0, cx0, W_ih, W_hh, b_ih, b_hh, W1, b1, W2, b2,
           steps=STEPS):
    args = [np.asarray(a, np.float32) for a in
            (x, hx0, cx0, W_ih, W_hh, b_ih, b_hh, W1, b1, W2, b2)]
    in_maps = _prep_host(*args)
    runner = _get_runner(steps)
    results = runner.run(in_maps)
    out = np.zeros((B, steps, OUT), np.float32)
    for ci in range(NCORES):
        yt = results[ci]["yt"]                   # [9, nt]
        out[BL * ci:BL * (ci + 1)] = (
            yt.reshape(OUT, steps, BL).transpose(2, 1, 0))
    kernel.last_in_maps = in_maps
    return out


# revision 13
# speedup vs baseline: 41.3322x; 41.3322x over previous
"""Trainium2 Bass kernel for nn_Decoder_44882408243654.

LSTMCell(H=512) recurrence, B=256, 326 steps where from step 1 on the cell
input IS the hidden state, plus a deferred MLP head (512->512 relu ->9).

Strategy:
- Data-parallel over batch: 8 cores x 32 rows, weights replicated, no
  collectives (the recurrence is strictly sequential in time).
- All weights + the full hidden-state history stay SBUF-resident; only 4 DMAs
  per core (three const blobs in, y out).
- Steps >=1 fold W_ih+W_hh into one matrix. Batch=32 gates matmul is packed
  4-wide into the PE array via tile_position col-tiling: PSUM gates tile is
  [128, 512] with partition groups = [o, i, f, g] gates.
- sigmoid(x) == (tanh(x/2)+1)/2: one tanh table set serves all activations;
  the +1/x0.5 algebra is folded into scalar_tensor_tensor ops and the
  doubled representations C2=2c, H2=2h (weight scaling folded on host).
- h is produced directly in transposed (lhsT) layout via PE transposes of o
  and c2, so the next step's matmul needs no extra data movement.
- The MLP head is batched over all 326 steps after the recurrence (full
  128-partition matmuls against the stored H2 history).
"""
import sys
sys.path.insert(0, "/opt/trn_rl_repo")

import numpy as np
import ml_dtypes
from contextlib import ExitStack

import concourse.bass as bass
import concourse.bacc as bacc_mod
import concourse.tile as tile
from concourse import mybir
from concourse.bass_utils import run_bass_kernel_spmd

F32 = mybir.dt.float32
BF16 = mybir.dt.bfloat16
AF = mybir.ActivationFunctionType
ALU = mybir.AluOpType

B, H, OUT = 256, 512, 9
STEPS = 326            # total outputs (1 initial + 325 recurrent)
NCORES = 8
BL = B // NCORES       # 32 rows per core
G4 = 4 * H             # 2048

# main bf16 blob column offsets
O_WCT = 0                      # 4 k-chunks x [128, 2048] (recurrent, 0.5*Wc)
O_BROW = O_WCT + 4 * G4        # row 0: bias row [2048]
O_ONES = O_BROW + G4           # row 0: ones [32]
O_W1T = O_ONES + 32            # W1 lhsT: 4 k-chunks x [128, 512]
O_W2T = O_W1T + 4 * 512        # W2 lhsT: 4 k-chunks x [128, 9] (16-col pad)
NB = O_W2T + 4 * 16

# step-0 bf16 blob (released after the recurrence)
S_WIH = 0                      # step-0 x weights, 4 x [128, 2048]
S_WHH = S_WIH + 4 * G4         # step-0 h weights
S_X0T = S_WHH + 4 * G4         # [128, 128] x transposed (lhsT layout)
S_H0T = S_X0T + 128            # [128, 128] hx0 transposed
NS = S_H0T + 128

# f32 blob column offsets
F_ID = 0                       # [128, 32] tiled identity(32)
F_C2 = 32                      # rows 64:96 = 2*cx0  [32, 512]
F_B1 = F_C2 + 512              # [128, 4] b1 per-partition cols per m-chunk
F_B2 = F_B1 + 4                # rows 0:9 col = b2
NF = F_B2 + 1

_BUILT = {}


def _build(steps, repeat=1):
    """Build + finalize the SPMD bass program for `steps` outputs.
    repeat>1 re-runs the whole computation (timing instrumentation)."""
    nt = steps * BL
    nc = bacc_mod.Bacc()
    d_bb = nc.declare_dram_parameter("blob_b", [128, NB], BF16, isOutput=False)
    d_s0 = nc.declare_dram_parameter("blob_s0", [128, NS], BF16, isOutput=False)
    d_bf = nc.declare_dram_parameter("blob_f", [128, NF], F32, isOutput=False)
    d_yt = nc.declare_dram_parameter("yt", [OUT, nt], F32, isOutput=True)

    with tile.TileContext(nc) as tc, ExitStack() as ctx:
        const = ctx.enter_context(tc.tile_pool(name="const", bufs=1))
        hist = ctx.enter_context(tc.tile_pool(name="hist", bufs=1))

        bb = const.tile([128, NB], BF16)
        bf = const.tile([128, NF], F32)
        nc.sync.dma_start(bb[:], d_bb[:])
        nc.sync.dma_start(bf[:], d_bf[:])

        wct = [bb[:, O_WCT + G4 * k:O_WCT + G4 * (k + 1)] for k in range(4)]
        brow = bb[0:1, O_BROW:O_BROW + G4]
        ones = bb[0:1, O_ONES:O_ONES + 32]
        w1t = [bb[:, O_W1T + 512 * k:O_W1T + 512 * (k + 1)] for k in range(4)]
        w2t = [bb[:, O_W2T + 16 * k:O_W2T + 16 * k + OUT] for k in range(4)]
        id_t = bf[:, F_ID:F_ID + 32]
        c2f = bf[:, F_C2:F_C2 + 512]     # rows 64:96 = C2 state (in place)
        b1c = bf[:, F_B1:F_B1 + 4]
        b2c = bf[0:OUT, F_B2:F_B2 + 1]

        HT = hist.tile([128, nt * 4], BF16)   # H2 history, lhsT layout

        # ---------------- recurrence ----------------
        with (
            tc.tile_pool(name="s0pool", bufs=1) as s0pool,
            tc.tile_pool(name="work", bufs=2) as work,
            tc.tile_pool(name="gps", bufs=2, space="PSUM") as gps,
            tc.tile_pool(name="tps", bufs=2, space="PSUM") as tps,
        ):
            s0 = s0pool.tile([128, NS], BF16)
            nc.sync.dma_start(s0[:], d_s0[:])
            wih = [s0[:, S_WIH + G4 * k:S_WIH + G4 * (k + 1)] for k in range(4)]
            whh = [s0[:, S_WHH + G4 * k:S_WHH + G4 * (k + 1)] for k in range(4)]
            x0t = s0[:, S_X0T:S_X0T + 128]
            h0t = s0[:, S_H0T:S_H0T + 128]

            tc.strict_bb_all_engine_barrier()

            for t in list(range(steps)) * repeat:
                gates = gps.tile([128, 512], F32, name="gates")
                # accumulation per col-group jg (gate order o,i,f,g):
                # bias row (K=1) then K-chunk matmuls
                if t == 0:
                    terms = [(x0t, wih), (h0t, whh)]
                else:
                    base = 128 * (t - 1)
                    hprev = HT[:, base:base + 128]
                    terms = [(hprev, wct)]
                for jg in range(4):
                    oap = gates[32 * jg:32 * jg + 32, :]
                    nc.tensor.matmul(oap, ones, brow[:, 512 * jg:512 * (jg + 1)],
                                     start=True, stop=False,
                                     tile_position=(0, 32 * jg))
                n_terms = len(terms)
                for ti, (lhs, rhs) in enumerate(terms):
                    for k in range(4):
                        last = (ti == n_terms - 1) and (k == 3)
                        for jg in range(4):
                            oap = gates[32 * jg:32 * jg + 32, :]
                            nc.tensor.matmul(
                                oap, lhs[:, 32 * k:32 * k + 32],
                                rhs[k][:, 512 * jg:512 * (jg + 1)],
                                start=False, stop=last,
                                tile_position=(0, 32 * jg))

                # activations: tanh(0.5 x) on o,i,f; tanh(x) on g (in place)
                t_sb = work.tile([96, 512], F32, name="t_sb")
                nc.scalar.activation(t_sb[:], gates[0:96, :], AF.Tanh,
                                     bias=0.0, scale=0.5)
                nc.scalar.activation(gates[96:128, :], gates[96:128, :], AF.Tanh)

                # u = (ti+1)*g  -> psum gates[32:64] (i rows dead)
                nc.vector.scalar_tensor_tensor(
                    gates[32:64, :], t_sb[32:64, :], 1.0, gates[96:128, :],
                    op0=ALU.add, op1=ALU.mult)
                # w = (tf+1)*C2 (both SBUF @base64)
                w_sb = work.tile([96, 512], F32, name="w_sb")
                nc.vector.scalar_tensor_tensor(
                    w_sb[64:96, :], t_sb[64:96, :], 1.0, c2f[64:96, :],
                    op0=ALU.add, op1=ALU.mult)
                # C2' = 0.5*w + u
                nc.vector.scalar_tensor_tensor(
                    c2f[64:96, :], w_sb[64:96, :], 0.5, gates[32:64, :],
                    op0=ALU.mult, op1=ALU.add)

                # transposes into PSUM: o rows (base 0), C2' (base 64)
                toT = tps.tile([128, 128], F32, name="toT")
                for j in range(4):
                    nc.tensor.transpose(toT[:, 32 * j:32 * j + 32],
                                        t_sb[0:32, 128 * j:128 * (j + 1)],
                                        id_t[0:32, :])
                c2T = tps.tile([128, 128], F32, name="c2T")
                for j in range(4):
                    nc.tensor.transpose(c2T[:, 32 * j:32 * j + 32],
                                        c2f[64:96, 128 * j:128 * (j + 1)],
                                        id_t[64:96, :])

                tcT = work.tile([128, 128], F32, name="tcT")
                nc.scalar.activation(tcT[:], c2T[:], AF.Tanh, bias=0.0, scale=0.5)
                # H2_t = (toT+1)*tcT  -> bf16 history slice (lhsT layout)
                nc.vector.scalar_tensor_tensor(
                    HT[:, 128 * t:128 * (t + 1)], toT[:], 1.0, tcT[:],
                    op0=ALU.add, op1=ALU.mult)

        # ---------------- batched MLP head ----------------
        # z.T = relu(0.5*W1 @ H2.T + b1)  [512, nt] ; y.T = W2 @ z.T + b2
        with (
            tc.tile_pool(name="ypool", bufs=1) as ypool,
            tc.tile_pool(name="zwork", bufs=2) as zwork,
            tc.tile_pool(name="zps", bufs=2, space="PSUM") as zpsp,
            tc.tile_pool(name="yps", bufs=2, space="PSUM") as ypsp,
        ):
            yT = ypool.tile([OUT, nt], F32)
            CT = 512           # columns (t,b) per tile = 16 time steps
            n_ct = (nt + CT - 1) // CT
            for ct in range(n_ct):
                c0 = ct * CT
                w = min(CT, nt - c0)
                tb0 = c0 // BL          # first t index in this tile
                ntb = w // BL           # t steps in this tile
                hblk = HT[:, 128 * tb0:128 * (tb0 + ntb)]
                hblk = hblk.rearrange("p (t x) -> p t x", x=128)
                z_sb = []
                for m in range(4):
                    zps = zpsp.tile([128, CT], F32, name="zps")
                    for k in range(4):
                        rhs = hblk[:, :, 32 * k:32 * k + 32]
                        nc.tensor.matmul(zps[:, 0:w],
                                         w1t[k][:, 128 * m:128 * (m + 1)],
                                         rhs, start=(k == 0), stop=(k == 3))
                    zt = zwork.tile([128, CT], BF16, name="z_sb", tag=f"z{m}")
                    # relu(x + b1), split between DVE and ACT to balance load
                    if m % 2 == 0:
                        nc.vector.tensor_scalar(
                            zt[:, 0:w], zps[:, 0:w], b1c[:, m:m + 1], 0.0,
                            ALU.add, ALU.max)
                    else:
                        nc.scalar.activation(zt[:, 0:w], zps[:, 0:w], AF.Relu,
                                             bias=b1c[:, m:m + 1], scale=1.0)
                    z_sb.append(zt)
                yps = ypsp.tile([OUT, CT], F32, name="yps")
                for k in range(4):
                    nc.tensor.matmul(yps[:, 0:w], w2t[k], z_sb[k][:, 0:w],
                                     start=(k == 0), stop=(k == 3))
                nc.scalar.activation(yT[0:OUT, c0:c0 + w], yps[:, 0:w],
                                     AF.Identity, bias=b2c, scale=1.0)

            nc.sync.dma_start(d_yt[:], yT[:])

    nc.finalize()
    return nc


def _prep_host(x, hx0, cx0, W_ih, W_hh, b_ih, b_hh, W1, b1, W2, b2):
    """Build the per-core input blobs (all weight algebra folded here)."""
    perm = [3, 0, 1, 2]  # pytorch gate blocks i,f,g,o -> device order o,i,f,g

    def reorder(wm):
        blocks = wm.reshape(4, H, -1) if wm.ndim == 2 else wm.reshape(4, H)
        return np.concatenate([blocks[p] for p in perm], axis=0)

    Wih_r = reorder(W_ih)            # [2048, 512]
    Whh_r = reorder(W_hh)
    bc_r = reorder(b_ih + b_hh)      # [2048]
    Wc_r = 0.5 * (Wih_r + Whh_r)     # input is H2=2h

    def kchunksT(Wm):  # -> [4, 128, 2048], rhs layout per k-chunk
        return np.stack([Wm[:, 128 * j:128 * (j + 1)].T for j in range(4)])

    blob_b = np.zeros((128, NB), np.float32)
    wctT = kchunksT(Wc_r)
    for k in range(4):
        blob_b[:, O_WCT + G4 * k:O_WCT + G4 * (k + 1)] = wctT[k]
    blob_b[0, O_BROW:O_BROW + G4] = bc_r
    blob_b[0, O_ONES:O_ONES + 32] = 1.0
    W1T = (0.5 * W1).T               # [512, 512]; input is H2=2h
    for k in range(4):
        blob_b[:, O_W1T + 512 * k:O_W1T + 512 * (k + 1)] = \
            W1T[128 * k:128 * (k + 1), :]
        blob_b[:, O_W2T + 16 * k:O_W2T + 16 * k + OUT] = \
            W2.T[128 * k:128 * (k + 1), :]
    blob_b = blob_b.astype(ml_dtypes.bfloat16)

    blob_s0 = np.zeros((128, NS), np.float32)
    wihT = kchunksT(Wih_r)
    whhT = kchunksT(Whh_r)
    for k in range(4):
        blob_s0[:, S_WIH + G4 * k:S_WIH + G4 * (k + 1)] = wihT[k]
        blob_s0[:, S_WHH + G4 * k:S_WHH + G4 * (k + 1)] = whhT[k]

    def lhsT128(mat):  # [32, 512] -> [128, 128] lhsT tile layout
        o = np.zeros((128, 128), np.float32)
        for j in range(4):
            o[:, 32 * j:32 * j + 32] = mat[:, 128 * j:128 * (j + 1)].T
        return o

    blob_f0 = np.zeros((128, NF), np.float32)
    blob_f0[:, F_ID:F_ID + 32] = np.tile(np.eye(32, dtype=np.float32), (4, 1))
    blob_f0[:, F_B1:F_B1 + 4] = b1.reshape(4, 128).T
    blob_f0[0:OUT, F_B2] = b2

    in_maps = []
    for ci in range(NCORES):
        sl = slice(BL * ci, BL * (ci + 1))
        s0c = blob_s0.copy()
        s0c[:, S_X0T:S_X0T + 128] = lhsT128(x[sl])
        s0c[:, S_H0T:S_H0T + 128] = lhsT128(hx0[sl])
        bfc = blob_f0.copy()
        bfc[64:96, F_C2:F_C2 + 512] = 2.0 * cx0[sl]
        in_maps.append({
            "blob_b": blob_b,
            "blob_s0": s0c.astype(ml_dtypes.bfloat16),
            "blob_f": bfc,
        })
    return in_maps


class _Runner:
    """Cached jit(shard_map(bass_exec)) runner, mirrors run_bass_via_pjrt
    but reusable across calls and benchable with device-resident inputs."""

    def __init__(self, nc):
        import jax
        from jax.sharding import Mesh, PartitionSpec, NamedSharding
        from jax.experimental.shard_map import shard_map
        from concourse import bass2jax, mybir as _mb
        bass2jax.install_neuronx_cc_hook()
        self.jax = jax
        self.nc = nc
        part_name = (nc.partition_id_tensor.name
                     if nc.partition_id_tensor else None)
        in_names, out_names, out_avals, zero_outs = [], [], [], []
        for alloc in nc.m.functions[0].allocations:
            if not isinstance(alloc, _mb.MemoryLocationSet):
                continue
            name = alloc.memorylocations[0].name
            if alloc.kind == "ExternalInput":
                if name != part_name:
                    in_names.append(name)
            elif alloc.kind == "ExternalOutput":
                out_names.append(name)
                shape = tuple(alloc.tensor_shape)
                dtype = _mb.dt.np(alloc.dtype)
                out_avals.append(jax.core.ShapedArray(shape, dtype))
                zero_outs.append(np.zeros(shape, dtype))
        self.in_names, self.out_names = in_names, out_names
        self.zero_outs = zero_outs
        all_names = in_names + out_names
        if part_name is not None:
            all_names = all_names + [part_name]
        all_names = tuple(all_names)

        def _bind(*operands):
            return bass2jax._bass_exec_p.bind(
                *operands, out_avals=tuple(out_avals), in_names=all_names,
                out_names=tuple(out_names),
                lowering_input_output_aliases=(),
                sim_require_finite=True, sim_require_nnan=True, nc=nc)

        self._bind = _bind

        def _body(*args):
            operands = list(args)
            if part_name is not None:
                operands.append(bass2jax.partition_id_tensor())
            return tuple(_bind(*operands))

        devices = jax.devices()[:NCORES]
        self.mesh = Mesh(np.asarray(devices), ("core",))
        spec = PartitionSpec("core")
        self.sharding = NamedSharding(self.mesh, spec)
        n_args = len(in_names) + len(out_names)
        self.fn = jax.jit(
            shard_map(_body, mesh=self.mesh, in_specs=(spec,) * n_args,
                      out_specs=(spec,) * len(out_names), check_rep=False),
            keep_unused=True)

    def _concat(self, in_maps):
        return [np.concatenate([np.asarray(m[n]) for m in in_maps], axis=0)
                for n in self.in_names] + \
               [np.zeros((NCORES * z.shape[0], *z.shape[1:]), z.dtype)
                for z in self.zero_outs]

    def run(self, in_maps):
        outs = self.fn(*self._concat(in_maps))
        res = []
        for c in range(NCORES):
            d = {}
            for i, n in enumerate(self.out_names):
                a = np.asarray(outs[i])
                d[n] = a.reshape(NCORES, a.shape[0] // NCORES, *a.shape[1:])[c]
            res.append(d)
        return res

    def bench(self, in_maps, iters=10):
        """Min wall time of the single-call jit with device-resident inputs
        and no output fetch. Differencing two step-count variants isolates
        device time from the ~73ms axon dispatch floor."""
        import time
        jax = self.jax
        dev_args = [jax.device_put(a, self.sharding)
                    for a in self._concat(in_maps)]
        jax.block_until_ready(dev_args)
        jax.block_until_ready(self.fn(*dev_args))  # warm
        best = float("inf")
        alltimes = []
        for _ in range(iters):
            t0 = time.perf_counter()
            jax.block_until_ready(self.fn(*dev_args))
            dt = time.perf_counter() - t0
            alltimes.append(dt)
            best = min(best, dt)
        return best, alltimes


_RUNNERS = {}


def _get_runner(steps, repeat=1):
    key = (steps, repeat)
    if key not in _RUNNERS:
        if key not in _BUILT:
            _BUILT[key] = _build(steps, repeat)
        _RUNNERS[key] = _Runner(_BUILT[key])
    return _RUNNERS[key]


def kernel(x, hx0, cx0, W_ih, W_hh, b_ih, b_hh, W1, b1, W2, b2,
           steps=STEPS):
    args = [np.asarray(a, np.float32) for a in
            (x, hx0, cx0, W_ih, W_hh, b_ih, b_hh, W1, b1, W2, b2)]
    in_maps = _prep_host(*args)
    runner = _get_runner(steps)
    results = runner.run(in_maps)
    kernel.last_runner = runner
    out = np.zeros((B, steps, OUT), np.float32)
    for ci in range(NCORES):
        yt = results[ci]["yt"]                   # [9, nt]
        out[BL * ci:BL * (ci + 1)] = (
            yt.reshape(OUT, steps, BL).transpose(2, 1, 0))
    kernel.last_in_maps = in_maps
    return out
